# revision 32
# baseline (speedup 1.0000x reference)
"""Conv1d (B=32, C_in=C_out=64, L=16384, K=3, VALID) on 8 trn2 cores.

Strategy: data-parallel over batch (4 batches/core) with polyphase-2
packing. The host splits each batch's signal into even/odd phases
stacked on 128 partitions (xP[0:64]=x[:,0::2], xP[64:128]=x[:,1::2]);
the conv then needs only TWO PSUM-accumulated matmuls per output tile
(vs 3 tap-matmuls unpacked) against block lhsT matrices
  A = [[W0^T, 0], [W1^T, W0^T]],  B = [[W2^T, W1^T], [0, W2^T]]
where pass B reads the same input tile shifted by one packed column.
Each packed output column holds 2 real output columns (even rows 0:64,
odd rows 64:128), so PE work per output column drops 3 -> 2 cycles and
the kernel is cleanly DMA-bound. Accumulation is fp32 in PSUM; I/O is
fp16 to halve HBM traffic (memory roofline). Bias fuses into the
PSUM->SBUF copy, split across ACT and DVE. Host does the (free)
polyphase pack/unpack. Shapes hardcoded from the spec.

DMA schedule: the whole per-core input AND output are SBUF-resident
(8 x 1MiB tiles each); all input DMAs then all output DMAs are queued
on the single in-order Sync HWDGE ring, so the input stream runs at
full single-stream HBM rate (~425 GB/s), compute follows input with no
buffer stalls (finishing ~10us before the ring's output phase ends),
and the buffered output then streams out at full rate. Output DRAM
rows are padded to 8192 cols so every output DMA row is 16KB-aligned.
A ~4.3us dense matmul warmup flips the PE HAM clock gate (4/8 -> 8/8)
before real matmuls start.
"""

import os

import numpy as np

from concourse import bacc, bass, mybir, tile
from concourse.bass_utils import run_bass_kernel_spmd

B, C, L, K = 32, 64, 16384, 3
LOUT = L - K + 1  # 16382
NCORES = 8
BPC = B // NCORES  # 4 batches per core
UNITS = BPC  # one polyphase unit per batch
P = 128  # partitions (2 phases x 64 ch)
T = L // 2  # 8192 packed input cols
U = LOUT // 2  # 8191 packed output cols
NJ = 512  # PSUM inner chunk (one fp32 bank)

F32 = mybir.dt.float32
F16 = mybir.dt.float16

CH = int(os.environ.get("CONV_CH", "4096"))
BUFS_IN = int(os.environ.get("CONV_BUFS_IN", "8"))
BUFS_OUT = int(os.environ.get("CONV_BUFS_OUT", "8"))
WARMUP = int(os.environ.get("CONV_WARMUP", "10"))
TAIL_RAW = int(os.environ.get("CONV_TAIL_RAW", "2"))

_NC_CACHE = []


def _chunk_lists():
    """Per-unit output-chunk schedules: 1 MiB chunks for full DMA stream
    efficiency. Every input and output chunk is SBUF-resident
    (BUFS >= chunk count), so nothing ever blocks on buffer recycling."""
    mid = [CH] * (U // CH) + ([U % CH] if U % CH else [])
    return [mid] * UNITS


def _build_nc():
    nc = bacc.Bacc("TRN2", target_bir_lowering=False, debug=False,
                   num_devices=NCORES)

    x2 = nc.dram_tensor("x2", [UNITS, P, T], F16, kind="ExternalInput")
    wT = nc.dram_tensor("wT", [P, 2, P], F16, kind="ExternalInput")
    b2 = nc.dram_tensor("b2", [P, 1], F32, kind="ExternalInput")
    # pad output rows to 8192 cols so every row starts 16KB-aligned in
    # HBM (the host drops the pad column)
    y2 = nc.dram_tensor("y2", [UNITS, P, U + 1], F16, kind="ExternalOutput")

    # raw (non-pool) SBUF buffers for the last TAIL_RAW output chunks —
    # they stay addressable after the TileContext exits
    tail_buf = None
    if TAIL_RAW:
        tail_buf = nc.sbuf_tensor("tail_buf", [P, TAIL_RAW, CH], F16).__enter__()

    with tile.TileContext(nc) as tc:
        with (
            tc.tile_pool(name="const", bufs=1) as const_pool,
            tc.tile_pool(name="inp", bufs=BUFS_IN) as inp_pool,
            tc.tile_pool(name="outp", bufs=BUFS_OUT) as outp_pool,
            tc.tile_pool(name="psum", bufs=8, space=bass.MemorySpace.PSUM)
            as psum_pool,
        ):
            # weights + bias ride the ACT HWDGE ring so the Sync ring's
            # first input chunk issues with zero head-of-line delay
            w = const_pool.tile([P, 2, P], F16)
            nc.scalar.dma_start(out=w[:], in_=wT[:])
            bias = const_pool.tile([P, 1], F32)
            nc.scalar.dma_start(out=bias[:], in_=b2[:])

            # HAM warm-up: the PE clock gate needs ~3.4us of DENSE matmul
            # activity to flip 4/8 -> 8/8, so run full-width dummy matmuls
            # back-to-back while the first input chunk is in flight; real
            # matmuls then start on a warm clock.
            if WARMUP:
                wz = const_pool.tile([P, NJ], F16)
                nc.vector.memset(wz[:], 0.0)
                for i in range(WARMUP):
                    wp = psum_pool.tile([P, NJ], F32, tag="acc",
                                        name=f"warm{i}")
                    nc.tensor.matmul(wp[:], wz[:, :P], wz[:],
                                     start=True, stop=True)

            # All input chunks + all output chunks are SBUF-resident
            # (BUFS >= chunk count), and input AND output DMAs share the
            # single in-order Sync HWDGE ring with every input emitted
            # first: the input stream gets strict bus priority and runs
            # at full single-stream HBM rate, compute follows with no
            # buffer stalls, and the buffered output then streams out at
            # full rate. This beats fair round-robin of in/out streams,
            # which lets the compute tail get exposed (and HAM-downclocked)
            # at the end.
            outs = []
            tail_outs = []
            chunk_list = _chunk_lists()
            n_chunks = sum(len(c) for c in chunk_list)
            ci = 0
            for u, chunks in enumerate(chunk_list):
                l0 = 0
                for n in chunks:
                    it = inp_pool.tile([P, CH + 1], F16, tag="in")
                    nc.sync.dma_start(out=it[:, :n + 1],
                                      in_=x2[u, :, l0:l0 + n + 1])
                    k = ci - (n_chunks - TAIL_RAW)
                    if k >= 0:
                        ot = tail_buf[:, k, :]
                    else:
                        ot = outp_pool.tile([P, CH], F16, tag="out")
                    ci += 1
                    for j0 in range(0, n, NJ):
                        nj = min(NJ, n - j0)
                        pt = psum_pool.tile([P, NJ], F32, tag="acc")
                        nc.tensor.matmul(pt[:, :nj], w[:, 0, :],
                                         it[:, j0:j0 + nj],
                                         start=True, stop=False)
                        nc.tensor.matmul(pt[:, :nj], w[:, 1, :],
                                         it[:, j0 + 1:j0 + 1 + nj],
                                         start=False, stop=True)
                        # psum -> sbuf with fused bias add, split across
                        # ACT and DVE so the bank frees twice as fast
                        h = nj // 2
                        nc.scalar.add(ot[:, j0:j0 + h], pt[:, :h],
                                      add=bias[:, 0:1])
                        nc.vector.tensor_scalar_add(ot[:, j0 + h:j0 + nj],
                                                    pt[:, h:nj],
                                                    bias[:, 0:1])
                    outs.append((u, l0, n, ot))
                    l0 += n
            # the last TAIL_RAW chunks ship via raw DMAs AFTER the
            # TileContext exits (see below)
            tail_outs = [(u, l0, n, k)
                         for k, (u, l0, n, _) in
                         enumerate(outs[n_chunks - TAIL_RAW:])]
            for u, l0, n, ot in outs[:n_chunks - TAIL_RAW]:
                nc.sync.dma_start(out=y2[u, :, l0:l0 + n], in_=ot[:, :n])

    # Raw epilogue: the TileContext exit barrier ended every other
    # engine's program, so their fixed NEFF semaphore-reset chains
    # (~6us, Tensor is the long pole) run overlapped with these final
    # output transfers instead of strictly after them. The tc-exit
    # barrier guarantees the source tiles are fully drained, and the
    # explicit sem wait keeps Sync's program open until the bytes land.
    if tail_outs:
        sem = nc.alloc_semaphore("tail_out_sem")
        for u, l0, n, k in tail_outs:
            nc.sync.dma_start(out=y2[u, :, l0:l0 + n],
                              in_=tail_buf[:, k, :n]).then_inc(sem, 16)
        nc.sync.wait_ge(sem, 16 * len(tail_outs))

    nc.compile()
    return nc


def _get_nc():
    if not _NC_CACHE:
        _NC_CACHE.append(_build_nc())
    return _NC_CACHE[0]


def _prep_weights(weight, bias):
    w0, w1, w2 = (np.ascontiguousarray(weight[:, :, k].T) for k in range(K))
    wT = np.zeros((P, 2, P), np.float32)
    wT[0:C, 0, 0:C] = w0
    wT[C:P, 0, 0:C] = w1
    wT[C:P, 0, C:P] = w0
    wT[0:C, 1, 0:C] = w2
    wT[0:C, 1, C:P] = w1
    wT[C:P, 1, C:P] = w2
    b2 = np.concatenate([bias, bias]).reshape(P, 1).astype(np.float32)
    return wT.astype(np.float16), b2


def kernel(x, weight, bias, _want_results=False, **run_kwargs):
    x = np.asarray(x, np.float32)
    weight = np.asarray(weight, np.float32)
    bias = np.asarray(bias, np.float32)
    nc = _get_nc()
    wT, b2 = _prep_weights(weight, bias)
    xP = np.empty((B, P, T), np.float16)
    xP[:, 0:C, :] = x[:, :, 0::2]
    xP[:, C:P, :] = x[:, :, 1::2]
    in_maps = [
        {"x2": np.ascontiguousarray(xP[BPC * i:BPC * (i + 1)]),
         "wT": wT, "b2": b2}
        for i in range(NCORES)
    ]
    res = run_bass_kernel_spmd(nc, in_maps, list(range(NCORES)), **run_kwargs)
    out = np.empty((B, C, LOUT), np.float32)
    for i in range(NCORES):
        yP = res.results[i]["y2"][:, :, :U].astype(np.float32)
        out[BPC * i:BPC * (i + 1), :, 0::2] = yP[:, 0:C, :]
        out[BPC * i:BPC * (i + 1), :, 1::2] = yP[:, C:P, :]
    if _want_results:
        return out, res
    return out


# revision 33
# speedup vs baseline: 1.1584x; 1.1584x over previous
"""Conv1d (B=32, C_in=C_out=64, L=16384, K=3, VALID) on 8 trn2 cores.

Strategy: data-parallel over batch (4 batches/core) with polyphase-2
packing. The host splits each batch's signal into even/odd phases
stacked on 128 partitions (xP[0:64]=x[:,0::2], xP[64:128]=x[:,1::2]);
the conv then needs only TWO PSUM-accumulated matmuls per output tile
(vs 3 tap-matmuls unpacked) against block lhsT matrices
  A = [[W0^T, 0], [W1^T, W0^T]],  B = [[W2^T, W1^T], [0, W2^T]]
where pass B reads the same input tile shifted by one packed column.
Each packed output column holds 2 real output columns (even rows 0:64,
odd rows 64:128), so PE work per output column drops 3 -> 2 cycles and
the kernel is cleanly DMA-bound. Accumulation is fp32 in PSUM; I/O is
fp16 to halve HBM traffic (memory roofline). Bias fuses into the
PSUM->SBUF copy, split across ACT and DVE. Host does the (free)
polyphase pack/unpack. Shapes hardcoded from the spec.

DMA schedule: the whole per-core input AND output are SBUF-resident
(8 x 1MiB tiles each); all input DMAs then all output DMAs are queued
on the single in-order Sync HWDGE ring, so the input stream runs at
full single-stream HBM rate (~425 GB/s), compute follows input with no
buffer stalls (finishing ~10us before the ring's output phase ends),
and the buffered output then streams out at full rate. Output DRAM
rows are padded to 8192 cols so every output DMA row is 16KB-aligned.
A ~4.3us dense matmul warmup flips the PE HAM clock gate (4/8 -> 8/8)
before real matmuls start.
"""

import os

import numpy as np

from concourse import bacc, bass, mybir, tile
from concourse.bass_utils import run_bass_kernel_spmd

B, C, L, K = 32, 64, 16384, 3
LOUT = L - K + 1  # 16382
NCORES = 8
BPC = B // NCORES  # 4 batches per core
UNITS = BPC  # one polyphase unit per batch
P = 128  # partitions (2 phases x 64 ch)
T = L // 2  # 8192 packed input cols
U = LOUT // 2  # 8191 packed output cols
NJ = 512  # PSUM inner chunk (one fp32 bank)

F32 = mybir.dt.float32
F16 = mybir.dt.float16

CH = int(os.environ.get("CONV_CH", "4096"))
BUFS_IN = int(os.environ.get("CONV_BUFS_IN", "8"))
BUFS_OUT = int(os.environ.get("CONV_BUFS_OUT", "8"))
WARMUP = int(os.environ.get("CONV_WARMUP", "10"))

_NC_CACHE = []


def _chunk_lists():
    """Per-unit output-chunk schedules: 1 MiB chunks for full DMA stream
    efficiency. Every input and output chunk is SBUF-resident
    (BUFS >= chunk count), so nothing ever blocks on buffer recycling."""
    mid = [CH] * (U // CH) + ([U % CH] if U % CH else [])
    return [mid] * UNITS


def _build_nc():
    nc = bacc.Bacc("TRN2", target_bir_lowering=False, debug=False,
                   num_devices=NCORES)

    x2 = nc.dram_tensor("x2", [UNITS, P, T], F16, kind="ExternalInput")
    wT = nc.dram_tensor("wT", [P, 2, P], F16, kind="ExternalInput")
    b2 = nc.dram_tensor("b2", [P, 1], F32, kind="ExternalInput")
    # pad output rows to 8192 cols so every row starts 16KB-aligned in
    # HBM (the host drops the pad column)
    y2 = nc.dram_tensor("y2", [UNITS, P, U + 1], F16, kind="ExternalOutput")

    with tile.TileContext(nc) as tc:
        with (
            tc.tile_pool(name="const", bufs=1) as const_pool,
            tc.tile_pool(name="inp", bufs=BUFS_IN) as inp_pool,
            tc.tile_pool(name="outp", bufs=BUFS_OUT) as outp_pool,
            tc.tile_pool(name="psum", bufs=8, space=bass.MemorySpace.PSUM)
            as psum_pool,
        ):
            # weights + bias ride the ACT HWDGE ring so the Sync ring's
            # first input chunk issues with zero head-of-line delay
            w = const_pool.tile([P, 2, P], F16)
            nc.scalar.dma_start(out=w[:], in_=wT[:])
            bias = const_pool.tile([P, 1], F32)
            nc.scalar.dma_start(out=bias[:], in_=b2[:])

            # HAM warm-up: the PE clock gate needs ~3.4us of DENSE matmul
            # activity to flip 4/8 -> 8/8, so run full-width dummy matmuls
            # back-to-back while the first input chunk is in flight; real
            # matmuls then start on a warm clock.
            if WARMUP:
                wz = const_pool.tile([P, NJ], F16)
                nc.vector.memset(wz[:], 0.0)
                for i in range(WARMUP):
                    wp = psum_pool.tile([P, NJ], F32, tag="acc",
                                        name=f"warm{i}")
                    nc.tensor.matmul(wp[:], wz[:, :P], wz[:],
                                     start=True, stop=True)

            # All input chunks + all output chunks are SBUF-resident
            # (BUFS >= chunk count), and input AND output DMAs share the
            # single in-order Sync HWDGE ring with every input emitted
            # first: the input stream gets strict bus priority and runs
            # at full single-stream HBM rate, compute follows with no
            # buffer stalls, and the buffered output then streams out at
            # full rate. This beats fair round-robin of in/out streams,
            # which lets the compute tail get exposed (and HAM-downclocked)
            # at the end.
            outs = []
            for u, chunks in enumerate(_chunk_lists()):
                l0 = 0
                for n in chunks:
                    it = inp_pool.tile([P, CH + 1], F16, tag="in")
                    nc.sync.dma_start(out=it[:, :n + 1],
                                      in_=x2[u, :, l0:l0 + n + 1])
                    ot = outp_pool.tile([P, CH], F16, tag="out")
                    for j0 in range(0, n, NJ):
                        nj = min(NJ, n - j0)
                        pt = psum_pool.tile([P, NJ], F32, tag="acc")
                        nc.tensor.matmul(pt[:, :nj], w[:, 0, :],
                                         it[:, j0:j0 + nj],
                                         start=True, stop=False)
                        nc.tensor.matmul(pt[:, :nj], w[:, 1, :],
                                         it[:, j0 + 1:j0 + 1 + nj],
                                         start=False, stop=True)
                        # psum -> sbuf with fused bias add, split across
                        # ACT and DVE so the bank frees twice as fast
                        h = nj // 2
                        nc.scalar.add(ot[:, j0:j0 + h], pt[:, :h],
                                      add=bias[:, 0:1])
                        nc.vector.tensor_scalar_add(ot[:, j0 + h:j0 + nj],
                                                    pt[:, h:nj],
                                                    bias[:, 0:1])
                    outs.append((u, l0, n, ot))
                    l0 += n
            for u, l0, n, ot in outs:
                nc.sync.dma_start(out=y2[u, :, l0:l0 + n], in_=ot[:, :n])

    nc.compile()
    return nc


def _get_nc():
    if not _NC_CACHE:
        _NC_CACHE.append(_build_nc())
    return _NC_CACHE[0]


def _prep_weights(weight, bias):
    w0, w1, w2 = (np.ascontiguousarray(weight[:, :, k].T) for k in range(K))
    wT = np.zeros((P, 2, P), np.float32)
    wT[0:C, 0, 0:C] = w0
    wT[C:P, 0, 0:C] = w1
    wT[C:P, 0, C:P] = w0
    wT[0:C, 1, 0:C] = w2
    wT[0:C, 1, C:P] = w1
    wT[C:P, 1, C:P] = w2
    b2 = np.concatenate([bias, bias]).reshape(P, 1).astype(np.float32)
    return wT.astype(np.float16), b2


def kernel(x, weight, bias, _want_results=False, **run_kwargs):
    x = np.asarray(x, np.float32)
    weight = np.asarray(weight, np.float32)
    bias = np.asarray(bias, np.float32)
    nc = _get_nc()
    wT, b2 = _prep_weights(weight, bias)
    xP = np.empty((B, P, T), np.float16)
    xP[:, 0:C, :] = x[:, :, 0::2]
    xP[:, C:P, :] = x[:, :, 1::2]
    in_maps = [
        {"x2": np.ascontiguousarray(xP[BPC * i:BPC * (i + 1)]),
         "wT": wT, "b2": b2}
        for i in range(NCORES)
    ]
    res = run_bass_kernel_spmd(nc, in_maps, list(range(NCORES)), **run_kwargs)
    out = np.empty((B, C, LOUT), np.float32)
    for i in range(NCORES):
        yP = res.results[i]["y2"][:, :, :U].astype(np.float32)
        out[BPC * i:BPC * (i + 1), :, 0::2] = yP[:, 0:C, :]
        out[BPC * i:BPC * (i + 1), :, 1::2] = yP[:, C:P, :]
    if _want_results:
        return out, res
    return out
